# revision 1
# baseline (speedup 1.0000x reference)
"""MetaGraphSAGE Trainium2 kernel (8 NeuronCores, Bass/Tile), v2.

Per metagraph (3 independent graphs):
    h  = ELU(mean_agg(x) @ W1l + x @ W1r + b1)
    o  = mean_agg(h @ W2l) + h @ W2r + b2
    out = log_softmax(o, axis=1)

Design vs v1:
- Layer 1 dst-partitioned: the 392 global 128-node dst blocks are
  bin-packed across the 8 cores per graph (greedy by edge count, slots
  sorted descending) so compile-time per-slot chunk counts (max over
  cores) hug the per-core actuals.
- Layer 2 src-partitioned: each core gathers P2 = h@W2l rows for its
  OWN nodes from local DRAM (no AllGather, single int16 segment),
  scatters partial sums over all 392 global dst blocks, then one bf16
  ReduceScatter(add) per graph delivers summed [NSH, D] rows to their
  owner core (pi position = owner*49 + slot).
- bf16 edge pipeline: x/P2 gathered as bf16, one-hot and all matmuls
  bf16 (PE 4x vs fp32), fp32 PSUM accumulate.
- ELU computed as relu(z) + exp(min(z,0)) (Act engine heavy); this is
  ELU+1, and the "-1" is folded into layer 2 constants:
  b2_eff = b2 - colsum(W2r) - colsum(W2l). Zero-in-degree nodes (for
  which the colsum(W2l) fold is wrong) are fixed up on the host.
- log_softmax without max-subtraction, exp row-sums via activation
  accum_out, ONE Ln per graph on the collected [128,49] sums (avoids
  per-block Exp<->Ln act-table thrash).
- tensor_scalar always dual-op (op1=min with +3e38): op1=bypass
  encoding measures 10-30x slower on HW.
- WIN=16 chunk gather calls (2048 idx) with 32KB SWDGE scratch; a
  global gather-call counter keeps gpool slot rotation == SWDGE queue.
"""

import sys

sys.path.insert(0, "/opt/trn_rl_repo")

import numpy as np
import ml_dtypes

BF16 = ml_dtypes.bfloat16

META, N, E, F, H, D = 3, 50000, 640000, 128, 128, 64
NCORES = 8
NBLK_G = 392          # global 128-node dst blocks (392*128 = 50176)
NSLOT = 49            # blocks per core
NSH = NSLOT * 128     # 6272
NPAD = NBLK_G * 128   # 50176
SPLIT = 32768         # int16 gather index limit
WIN = 8               # 128-edge chunks per dma_gather call (1024 idx HW cap)
GRP = 6               # L1 slots per psum group (12 regions -> 3 banks)
GRP2 = 7              # L2 dst blocks per psum tile (49=7x7: groups never
                      # span core boundaries of the core-major partial buffer)
DMA_SCRATCH = 32768   # per-partition SWDGE descriptor carveout bytes
NQUEUE = 4
PHASE = 3             # debug: 1 = L1 only, 3 = full


def _ceil(a, b):
    return (a + b - 1) // b


def _wrap_idx(a):
    # idx i -> [i%16, i//16], replicated to 128 partitions
    return np.tile(a.reshape(-1, 16).T, (8, 1))


def _wrap_dst(a):
    return a.reshape(-1, 128).T.copy()


def _runpos(keys):
    """Position of each element within its run of equal consecutive keys."""
    n = len(keys)
    if n == 0:
        return np.zeros((0,), dtype=np.int64)
    change = np.r_[True, keys[1:] != keys[:-1]]
    runstart = np.maximum.accumulate(np.where(change, np.arange(n), 0))
    return np.arange(n) - runstart


def _prep_host(meta_x, meta_edge_index):
    meta_x = np.asarray(meta_x, dtype=np.float32)
    ei = np.asarray(meta_edge_index, dtype=np.int64)

    xb = meta_x.astype(BF16)  # [META, N, F] L1 gather source (shared)

    inv_all = np.zeros((META, NPAD), dtype=np.float32)
    for g in range(META):
        cnt = np.bincount(ei[g, 1], minlength=NPAD).astype(np.float32)
        inv_all[g] = 1.0 / np.maximum(cnt, 1.0)

    # --- bin-pack global dst blocks to (core, slot) per graph ---
    owner = np.zeros((META, NBLK_G), dtype=np.int64)
    slot = np.zeros((META, NBLK_G), dtype=np.int64)
    slots_of = np.zeros((META, NCORES, NSLOT), dtype=np.int64)
    for g in range(META):
        w = np.bincount(ei[g, 1] >> 7, minlength=NBLK_G)
        order = np.argsort(-w, kind="stable")
        loads = [0] * NCORES
        counts = [0] * NCORES
        per_core_blocks = [[] for _ in range(NCORES)]
        for b in order:
            c = min(
                (c for c in range(NCORES) if counts[c] < NSLOT),
                key=lambda c: loads[c],
            )
            per_core_blocks[c].append(b)
            loads[c] += w[b]
            counts[c] += 1
        for c in range(NCORES):
            for s, b in enumerate(per_core_blocks[c]):
                owner[g, b] = c
                slot[g, b] = s
                slots_of[g, c, s] = b
    pos = owner * NSLOT + slot  # [META, NBLK_G] global pi position

    # --- L1 edges: partition by dst-block owner, sort by (slot, seg) ---
    cnt1 = np.zeros((NCORES, META, NSLOT, 2), dtype=np.int64)
    e1 = {}
    for g in range(META):
        src, dst = ei[g, 0], ei[g, 1]
        bd = dst >> 7
        oc = owner[g, bd]
        sl = slot[g, bd]
        seg = (src >= SPLIT).astype(np.int64)
        key = sl * 2 + seg
        for c in range(NCORES):
            m = oc == c
            k, s, d = key[m], src[m], dst[m]
            o = np.argsort(k, kind="stable")
            k, s, d = k[o], s[o], d[o]
            cnt1[c, g] = np.bincount(k, minlength=NSLOT * 2).reshape(NSLOT, 2)
            idx = np.where(k % 2 == 1, s - SPLIT, s).astype(np.int16)
            e1[(c, g)] = (k, idx, (d & 127).astype(np.float32))

    nch1 = _ceil(cnt1.max(axis=0), 128)  # [META, NSLOT, 2]

    # stream1[g]: (slot, seg) per chunk; per group: [lo section | hi section]
    stream1 = [[] for _ in range(META)]
    sec1 = [[] for _ in range(META)]  # per g: (seg, chunk0, nchunk) x2 per grp
    for g in range(META):
        for s0 in range(0, NSLOT, GRP):
            ns = min(GRP, NSLOT - s0)
            for seg in (0, 1):
                c0 = len(stream1[g])
                for s in range(s0, s0 + ns):
                    stream1[g] += [(s, seg)] * int(nch1[g, s, seg])
                sec1[g].append((seg, c0, len(stream1[g]) - c0))
    totc1 = [len(stream1[g]) for g in range(META)]

    # --- L2 edges: partition by src-block owner, sort by dst pi position ---
    cnt2 = np.zeros((NCORES, META, NBLK_G), dtype=np.int64)
    e2 = {}
    for g in range(META):
        src, dst = ei[g, 0], ei[g, 1]
        bs = src >> 7
        oc = owner[g, bs]
        p = pos[g, dst >> 7]
        lrow = slot[g, bs] * 128 + (src & 127)
        for c in range(NCORES):
            m = oc == c
            pp, lr, d = p[m], lrow[m], dst[m]
            o = np.argsort(pp, kind="stable")
            pp, lr, d = pp[o], lr[o], d[o]
            cnt2[c, g] = np.bincount(pp, minlength=NBLK_G)
            e2[(c, g)] = (pp, lr.astype(np.int16), (d & 127).astype(np.float32))

    nch2 = _ceil(cnt2.max(axis=0), 128)  # [META, NBLK_G]
    stream2 = [[] for _ in range(META)]
    sec2 = [[] for _ in range(META)]  # per g: (p0, np, chunk0, nchunk)
    for g in range(META):
        for p0 in range(0, NBLK_G, GRP2):
            np_ = min(GRP2, NBLK_G - p0)
            c0 = len(stream2[g])
            for p in range(p0, p0 + np_):
                stream2[g] += [p] * int(nch2[g, p])
            sec2[g].append((p0, np_, c0, len(stream2[g]) - c0))
    totc2 = [len(stream2[g]) for g in range(META)]

    # --- per-core flat arrays in stream order ---
    per_core = []
    for c in range(NCORES):
        i1 = [np.zeros((t * 128,), dtype=np.int16) for t in totc1]
        d1 = [np.full((t * 128,), -1.0, dtype=np.float32) for t in totc1]
        i2 = [np.zeros((t * 128,), dtype=np.int16) for t in totc2]
        d2 = [np.full((t * 128,), -1.0, dtype=np.float32) for t in totc2]
        for g in range(META):
            off_arr = np.full((NSLOT * 2,), -1, dtype=np.int64)
            for ci, (s_, seg_) in enumerate(stream1[g]):
                k_ = s_ * 2 + seg_
                if off_arr[k_] < 0:
                    off_arr[k_] = ci * 128
            k1, idx, d128 = e1[(c, g)]
            tgt = off_arr[k1] + _runpos(k1)
            i1[g][tgt] = idx
            d1[g][tgt] = d128

            off2_arr = np.full((NBLK_G,), -1, dtype=np.int64)
            for ci, p_ in enumerate(stream2[g]):
                if off2_arr[p_] < 0:
                    off2_arr[p_] = ci * 128
            p2, lr, dd = e2[(c, g)]
            tgt = off2_arr[p2] + _runpos(p2)
            i2[g][tgt] = lr
            d2[g][tgt] = dd

        idx1 = np.concatenate([_wrap_idx(a) for a in i1], axis=1)
        dst1 = np.concatenate([_wrap_dst(a) for a in d1], axis=1).astype(BF16)
        idx2 = np.concatenate([_wrap_idx(a) for a in i2], axis=1)
        dst2 = np.concatenate([_wrap_dst(a) for a in d2], axis=1).astype(BF16)

        xts = np.zeros((META, 128, NSH), dtype=BF16)
        invb = np.zeros((META, 128, NSH), dtype=BF16)
        invt = np.zeros((META, 128, NSLOT), dtype=np.float32)
        for g in range(META):
            blocks = slots_of[g, c]
            rows = (blocks[:, None] * 128 + np.arange(128)[None, :]).reshape(-1)
            valid = rows < N
            xg = np.zeros((NSH, F), dtype=np.float32)
            xg[valid] = meta_x[g][rows[valid]]
            xts[g] = xg.T.astype(BF16)
            inv_rows = inv_all[g][rows]
            invb[g] = np.broadcast_to(inv_rows[None, :], (128, NSH)).astype(BF16)
            invt[g] = inv_rows.reshape(NSLOT, 128).T
        per_core.append(
            dict(idx1=idx1, dst1=dst1, idx2=idx2, dst2=dst2,
                 xts=xts, invb=invb, invt=invt)
        )

    secmax1 = max(max((n for (_, _, n) in sec1[g]), default=1) for g in range(META))
    secmax2 = max(max((n for (_, _, _, n) in sec2[g]), default=1) for g in range(META))
    layout = dict(
        stream1=stream1, sec1=sec1, totc1=totc1, nch1=nch1,
        stream2=stream2, sec2=sec2, totc2=totc2, nch2=nch2,
        slots_of=slots_of, secmax1=secmax1, secmax2=secmax2,
    )
    return layout, per_core, xb


def _build_program(layout):
    import concourse.mybir as mybir
    import concourse.tile as tile
    from concourse import bacc

    fp32 = mybir.dt.float32
    bf16 = mybir.dt.bfloat16
    i16 = mybir.dt.int16
    AF = mybir.ActivationFunctionType
    OP = mybir.AluOpType

    nc = bacc.Bacc(None, dynamic_dma_scratch_size=DMA_SCRATCH,
                   num_swdge_queues=NQUEUE)
    core_ids = list(range(NCORES))

    stream1, sec1, totc1, nch1 = (
        layout["stream1"], layout["sec1"], layout["totc1"], layout["nch1"])
    stream2, sec2, totc2, nch2 = (
        layout["stream2"], layout["sec2"], layout["totc2"], layout["nch2"])
    secmax1, secmax2 = layout["secmax1"], layout["secmax2"]
    T1, T2 = sum(totc1), sum(totc2)
    goff1 = [sum(totc1[:g]) for g in range(META)]
    goff2 = [sum(totc2[:g]) for g in range(META)]

    xb_in = nc.declare_dram_parameter("xb", [META, N, F], bf16, isOutput=False)
    idx1_in = nc.declare_dram_parameter("idx1", [128, T1 * 8], i16, isOutput=False)
    dst1_in = nc.declare_dram_parameter("dst1", [128, T1], bf16, isOutput=False)
    idx2_in = nc.declare_dram_parameter("idx2", [128, T2 * 8], i16, isOutput=False)
    dst2_in = nc.declare_dram_parameter("dst2", [128, T2], bf16, isOutput=False)
    xts_in = nc.declare_dram_parameter("xts", [META, 128, NSH], bf16, isOutput=False)
    invb_in = nc.declare_dram_parameter("invb", [META, 128, NSH], bf16, isOutput=False)
    invt_in = nc.declare_dram_parameter("invt", [META, 128, NSLOT], fp32, isOutput=False)
    w1l_in = nc.declare_dram_parameter("w1l", [META, F, H], bf16, isOutput=False)
    w1r_in = nc.declare_dram_parameter("w1r", [META, F, H], bf16, isOutput=False)
    b1_in = nc.declare_dram_parameter("b1c", [META, H, 1], fp32, isOutput=False)
    w2lp_in = nc.declare_dram_parameter("w2lp", [META, H, 128], bf16, isOutput=False)
    w2r_in = nc.declare_dram_parameter("w2r", [META, H, D], bf16, isOutput=False)
    b2e_in = nc.declare_dram_parameter("b2e", [META, 1, D], bf16, isOutput=False)
    ones_in = nc.declare_dram_parameter("ones1", [1, 128], bf16, isOutput=False)
    iota_in = nc.declare_dram_parameter("iota", [128, 128], bf16, isOutput=False)
    out_ext = nc.declare_dram_parameter("out", [META, 128, NSLOT * D], fp32, isOutput=True)

    p2sh = [nc.dram_tensor(f"p2sh{g}", [NSH, 128], bf16) for g in range(META)]
    part = [
        nc.dram_tensor(f"part{g}", [NCORES, 128, NSLOT * D], bf16)
        for g in range(META)
    ]
    l2r = [
        nc.dram_tensor(f"l2r{g}", [128, NSLOT * D], bf16)
        for g in range(META)
    ]

    with tile.TileContext(nc) as tc:
        with (
            tc.tile_pool(name="const", bufs=1) as cpool,
            tc.tile_pool(name="weights", bufs=1) as wpool,
            tc.tile_pool(name="hblk", bufs=1) as hpool,
            tc.tile_pool(name="gath", bufs=8) as gpool,
            tc.tile_pool(name="oneh", bufs=8) as opool,
            tc.tile_pool(name="meta", bufs=4) as mpool,
            tc.tile_pool(name="dense", bufs=4) as dpool,
            tc.tile_pool(name="fin", bufs=1) as fpool,
            tc.tile_pool(name="psA", bufs=1, space="PSUM") as psA,
            tc.tile_pool(name="psB", bufs=2, space="PSUM") as psB,
            tc.tile_pool(name="psC", bufs=1, space="PSUM") as psC,
            tc.tile_pool(name="psD", bufs=2, space="PSUM") as psD,
        ):
            iota_t = cpool.tile([128, 128], bf16, tag="iota", name="iota_t")
            nc.sync.dma_start(out=iota_t[:], in_=iota_in[:])
            ones_t = cpool.tile([1, 128], bf16, tag="ones1", name="ones_t")
            nc.sync.dma_start(out=ones_t[:], in_=ones_in[:])

            w1l_t, w1r_t, w2lp_t, w2r_t, b1_t, invt_t, b2e_t = (
                [], [], [], [], [], [], [])
            for g in range(META):
                for lst, src_ap, shp, dt, nm in (
                    (w1l_t, w1l_in[g], [F, H], bf16, "w1l"),
                    (w1r_t, w1r_in[g], [F, H], bf16, "w1r"),
                    (w2lp_t, w2lp_in[g], [H, 128], bf16, "w2lp"),
                    (w2r_t, w2r_in[g], [H, D], bf16, "w2r"),
                    (b1_t, b1_in[g], [H, 1], fp32, "b1"),
                    (invt_t, invt_in[g], [128, NSLOT], fp32, "invt"),
                    (b2e_t, b2e_in[g], [1, D], bf16, "b2e"),
                ):
                    t = wpool.tile(shp, dt, tag=f"{nm}{g}", name=f"{nm}{g}")
                    nc.sync.dma_start(out=t[:], in_=src_ap)
                    lst.append(t)

            hblk = {}
            qctr = [0]  # global gather-call counter == gpool slot rotation

            def gather_call(src_ap, idxt, lc, wn, tagname):
                gt = gpool.tile([128, WIN, 128], bf16, tag="gt", name=tagname)
                nc.gpsimd.dma_gather(
                    gt[:, :wn, :],
                    src_ap,
                    idxt[:, lc * 8: (lc + wn) * 8],
                    wn * 128,
                    wn * 128,
                    128,
                    queue_num=qctr[0] % NQUEUE,
                )
                qctr[0] += 1
                return gt

            def onehot(dstt, lc, wn):
                oh = opool.tile([128, WIN, 128], bf16, tag="oh", name="oh")
                nc.vector.tensor_tensor(
                    out=oh[:, :wn, :],
                    in0=dstt[:, lc: lc + wn]
                    .rearrange("p (w o) -> p w o", o=1)
                    .to_broadcast([128, wn, 128]),
                    in1=iota_t[:]
                    .rearrange("p (o d) -> p o d", o=1)
                    .to_broadcast([128, wn, 128]),
                    op=OP.is_equal,
                )
                return oh

            # ============ phases (interleaved to keep the in-order Pool
            # engine fed: L2(g) waits on p2sh(g), so L1(g+1) is issued
            # before it) =====================================================
            def do_L1(g):
                ngroups = _ceil(NSLOT, GRP)
                for gi in range(ngroups):
                    s0 = gi * GRP
                    ns = min(GRP, NSLOT - s0)
                    nbg = ns * 128
                    ibg = mpool.tile([128, GRP * 128], bf16, tag="ibg", name="ibg")
                    nc.sync.dma_start(
                        out=ibg[:, :nbg],
                        in_=invb_in[g, :, s0 * 128: s0 * 128 + nbg],
                    )
                    xtg = mpool.tile([128, GRP * 128], bf16, tag="xtg", name="xtg")
                    nc.sync.dma_start(
                        out=xtg[:, :nbg],
                        in_=xts_in[g, :, s0 * 128: s0 * 128 + nbg],
                    )
                    ps = [
                        psA.tile([128, 512], fp32, tag=f"edge{i}", name=f"ps{i}")
                        for i in range(3)
                    ]
                    p2g = mpool.tile([128, GRP * 128], bf16, tag="p2g",
                                     name="p2g")

                    def psl(s, seg, ps=ps, s0=s0):
                        j = (s - s0) * 2 + seg
                        return ps[j // 4][:, (j % 4) * 128: (j % 4) * 128 + 128]

                    left = {}
                    left_tot = {}
                    for s in range(s0, s0 + ns):
                        left_tot[s] = int(nch1[g, s, 0] + nch1[g, s, 1])
                        for seg in (0, 1):
                            left[(s, seg)] = int(nch1[g, s, seg])
                    started = set()

                    def drain_slot(s, ibg=ibg, xtg=xtg, psl=psl, s0=s0, g=g):
                        boff = (s - s0) * 128
                        n_lo = int(nch1[g, s, 0])
                        n_hi = int(nch1[g, s, 1])
                        m1 = dpool.tile([128, 128], bf16, tag="m1", name="m1")
                        if n_lo and n_hi:
                            s0t = dpool.tile([128, 128], bf16, tag="s0",
                                             name="s0t")
                            nc.vector.tensor_copy(out=s0t[:], in_=psl(s, 0))
                            s1t = dpool.tile([128, 128], bf16, tag="s1",
                                             name="s1t")
                            nc.vector.tensor_tensor(
                                out=s1t[:], in0=psl(s, 1), in1=s0t[:],
                                op=OP.add,
                            )
                            nc.vector.tensor_tensor(
                                out=m1[:], in0=s1t[:],
                                in1=ibg[:, boff: boff + 128], op=OP.mult,
                            )
                        elif n_lo or n_hi:
                            nc.vector.tensor_tensor(
                                out=m1[:], in0=psl(s, 0 if n_lo else 1),
                                in1=ibg[:, boff: boff + 128], op=OP.mult,
                            )
                        else:
                            nc.vector.memset(m1[:], 0.0)
                        o1 = psB.tile([H, 128], fp32, tag="work", name="o1")
                        nc.tensor.matmul(
                            out=o1[:], lhsT=w1l_t[g][:], rhs=m1[:],
                            start=True, stop=False,
                        )
                        nc.tensor.matmul(
                            out=o1[:], lhsT=w1r_t[g][:],
                            rhs=xtg[:, boff: boff + 128],
                            start=False, stop=True,
                        )
                        # ELU+1 = relu(z) + exp(min(z,0)); z = o1 + b1
                        tm = dpool.tile([H, 128], bf16, tag="tm", name="tm")
                        nc.vector.tensor_scalar(
                            out=tm[:], in0=o1[:],
                            scalar1=b1_t[g][:, :1], scalar2=0.0,
                            op0=OP.add, op1=OP.min,
                        )
                        te = dpool.tile([H, 128], bf16, tag="te", name="te")
                        nc.scalar.activation(out=te[:], in_=tm[:], func=AF.Exp)
                        tp = dpool.tile([H, 128], bf16, tag="tp", name="tp")
                        nc.scalar.activation(
                            out=tp[:], in_=o1[:], func=AF.Relu,
                            bias=b1_t[g][:, :1],
                        )
                        hb = hpool.tile([H, 128], bf16, tag=f"h{g}_{s}",
                                        name=f"h{g}_{s}")
                        nc.vector.tensor_tensor(
                            out=hb[:], in0=te[:], in1=tp[:], op=OP.add
                        )
                        hblk[(g, s)] = hb
                        p2p = psB.tile([128, 128], fp32, tag="work", name="p2p")
                        nc.tensor.matmul(
                            out=p2p[:], lhsT=hb[:], rhs=w2lp_t[g][:],
                            start=True, stop=True,
                        )
                        nc.vector.tensor_copy(
                            out=p2g[:, boff: boff + 128], in_=p2p[:]
                        )

                    for (seg, c0, ncols) in sec1[g][gi * 2: gi * 2 + 2]:
                        if ncols == 0:
                            continue
                        src_ap = (xb_in[g, :, :] if seg == 0
                                  else xb_in[g, SPLIT:, :])
                        gcol = goff1[g] + c0
                        idxt = mpool.tile([128, secmax1 * 8], i16,
                                          tag="idx1", name="idxt")
                        nc.sync.dma_start(
                            out=idxt[:, : ncols * 8],
                            in_=idx1_in[:, gcol * 8: (gcol + ncols) * 8],
                        )
                        dstt = mpool.tile([128, secmax1], bf16,
                                          tag="dst1", name="dstt")
                        nc.sync.dma_start(
                            out=dstt[:, :ncols],
                            in_=dst1_in[:, gcol: gcol + ncols],
                        )
                        for w0 in range(0, ncols, WIN):
                            wn = min(WIN, ncols - w0)
                            gt = gather_call(src_ap, idxt, w0, wn, "gt1")
                            oh = onehot(dstt, w0, wn)
                            for j in range(wn):
                                s, sg = stream1[g][c0 + w0 + j]
                                first = (s, sg) not in started
                                if first:
                                    started.add((s, sg))
                                left[(s, sg)] -= 1
                                left_tot[s] -= 1
                                nc.tensor.matmul(
                                    out=psl(s, sg),
                                    lhsT=gt[:, j, :],
                                    rhs=oh[:, j, :],
                                    start=first,
                                    stop=left[(s, sg)] == 0,
                                    skip_group_check=True,
                                )
                                if left_tot[s] == 0:
                                    drain_slot(s)
                    # slots with zero chunks in both segs
                    for s in range(s0, s0 + ns):
                        if left_tot[s] == 0 and (g, s) not in hblk:
                            drain_slot(s)
                    nc.sync.dma_start(
                        out=p2sh[g][s0 * 128: s0 * 128 + nbg, :]
                        .rearrange("(w p) d -> p w d", p=128),
                        in_=p2g[:, :nbg].rearrange("p (w d) -> p w d", d=128),
                    )

            def do_L2(g):
                for (p0, np_, c0, ncols) in sec2[g]:
                    if ncols == 0:
                        continue
                    psd = psD.tile([128, GRP2 * 64], fp32, tag="l2", name="psd")
                    left2 = {}
                    for p in range(p0, p0 + np_):
                        left2[p] = int(nch2[g, p])
                        if left2[p] == 0:
                            r = p - p0
                            nc.vector.memset(psd[:, r * 64: r * 64 + 64], 0.0)
                    started2 = set()
                    gcol = goff2[g] + c0
                    idxt = mpool.tile([128, secmax2 * 8], i16,
                                      tag="idx2", name="idxt2")
                    nc.sync.dma_start(
                        out=idxt[:, : ncols * 8],
                        in_=idx2_in[:, gcol * 8: (gcol + ncols) * 8],
                    )
                    dstt = mpool.tile([128, secmax2], bf16,
                                      tag="dst2", name="dstt2")
                    nc.sync.dma_start(
                        out=dstt[:, :ncols],
                        in_=dst2_in[:, gcol: gcol + ncols],
                    )
                    l2win = [(w0, min(WIN, ncols - w0))
                             for w0 in range(0, ncols, WIN)]
                    for w0, wn in l2win:
                        gt = gather_call(p2sh[g][:, :], idxt, w0, wn, "gt2")
                        oh = onehot(dstt, w0, wn)
                        for j in range(wn):
                            p = stream2[g][c0 + w0 + j]
                            first = p not in started2
                            if first:
                                started2.add(p)
                            left2[p] -= 1
                            r = p - p0
                            nc.tensor.matmul(
                                out=psd[:, r * 64: r * 64 + 64],
                                lhsT=oh[:, j, :],
                                rhs=gt[:, j, :64],
                                start=first,
                                stop=left2[p] == 0,
                                skip_group_check=True,
                            )

                    pp = dpool.tile([128, GRP2 * 64], bf16, tag="pp",
                                    name="pp")
                    nc.vector.tensor_copy(
                        out=pp[:, : np_ * 64], in_=psd[:, : np_ * 64]
                    )
                    cown = p0 // NSLOT
                    soff = (p0 - cown * NSLOT) * 64
                    nc.sync.dma_start(
                        out=part[g][cown, :, soff: soff + np_ * 64],
                        in_=pp[:, : np_ * 64],
                    )

                nc.gpsimd.collective_compute(
                    "ReduceScatter",
                    mybir.AluOpType.add,
                    ins=[part[g][:]],
                    outs=[l2r[g][:]],
                    replica_groups=[core_ids],
                )

            def do_final(g):
                smT = fpool.tile([128, NSLOT], fp32, tag=f"sm{g}", name=f"sm{g}")
                agT = fpool.tile([128, NSLOT * D], bf16, tag="agT", name="agT")
                nc.sync.dma_start(out=agT[:], in_=l2r[g][:])
                obT = fpool.tile([128, NSLOT * D], fp32, tag="obT", name="obT")
                t3s = []
                for s in range(NSLOT):
                    ag = agT[:, s * D: s * D + D]
                    o2 = psC.tile([128, D], fp32, tag="o2", name="o2")
                    nc.tensor.matmul(
                        out=o2[:], lhsT=hblk[(g, s)][:], rhs=w2r_t[g][:],
                        start=True, stop=False,
                    )
                    nc.tensor.matmul(
                        out=o2[:], lhsT=ones_t[:1, :], rhs=b2e_t[g][:1, :],
                        start=False, stop=True,
                    )
                    t3 = fpool.tile([128, D], fp32, tag=f"t3_{s}", name=f"t3_{s}")
                    nc.vector.scalar_tensor_tensor(
                        out=t3[:], in0=ag,
                        scalar=invt_t[g][:, s: s + 1],
                        in1=o2[:], op0=OP.mult, op1=OP.add,
                    )
                    t3s.append(t3)
                    ex = dpool.tile([128, D], bf16, tag="ex", name="ex")
                    nc.scalar.activation(
                        out=ex[:], in_=t3[:], func=AF.Exp,
                        accum_out=smT[:, s: s + 1],
                    )
                ln49 = fpool.tile([128, NSLOT], fp32, tag=f"ln{g}", name=f"ln{g}")
                nc.scalar.activation(out=ln49[:], in_=smT[:], func=AF.Ln)
                for s in range(NSLOT):
                    nc.vector.tensor_scalar(
                        out=obT[:, s * D: s * D + D], in0=t3s[s][:],
                        scalar1=ln49[:, s: s + 1], scalar2=3.0e38,
                        op0=OP.subtract, op1=OP.min,
                    )
                nc.sync.dma_start(out=out_ext[g], in_=obT[:])

            do_L1(0)
            do_L1(1)
            if PHASE != 1:
                do_L2(0)
            do_L1(2)
            if PHASE != 1:
                do_L2(1)
                do_L2(2)
                do_final(0)
                do_final(1)
                do_final(2)

    nc.finalize()
    return nc


def kernel(**inputs):
    out, _ = run_kernel(inputs)
    return out


def run_kernel(inputs, trace=False):
    from concourse.bass_utils import run_bass_kernel_spmd

    meta_x = np.asarray(inputs["meta_x"], dtype=np.float32)
    ei = np.asarray(inputs["meta_edge_index"], dtype=np.int64)
    layout, per_core, xb = _prep_host(meta_x, ei)
    nc = _build_program(layout)

    w1l = np.asarray(inputs["W1l"], dtype=np.float32)
    w1r = np.asarray(inputs["W1r"], dtype=np.float32)
    w2l = np.asarray(inputs["W2l"], dtype=np.float32)
    w2r = np.asarray(inputs["W2r"], dtype=np.float32)
    b1 = np.asarray(inputs["b1"], dtype=np.float32)
    b2 = np.asarray(inputs["b2"], dtype=np.float32)

    w2lp = np.zeros((META, H, 128), dtype=BF16)
    w2lp[:, :, :D] = w2l.astype(BF16)
    # "-1" fold of ELU+1: subtract colsum(W2r) (h path) and colsum(W2l)
    # (mean path; wrong only for zero-in-degree nodes, host-fixed below).
    b2e = (b2 - w2r.sum(axis=1) - w2l.sum(axis=1))[:, None, :].astype(BF16)
    iota = np.broadcast_to(
        np.arange(128, dtype=np.float32)[None, :], (128, 128)
    ).astype(BF16)
    ones1 = np.ones((1, 128), dtype=BF16)

    in_maps = []
    for c in range(NCORES):
        pc = per_core[c]
        in_maps.append(
            {
                "xb": xb,
                "idx1": pc["idx1"], "dst1": pc["dst1"],
                "idx2": pc["idx2"], "dst2": pc["dst2"],
                "xts": pc["xts"], "invb": pc["invb"], "invt": pc["invt"],
                "w1l": w1l.astype(BF16), "w1r": w1r.astype(BF16),
                "b1c": b1[:, :, None].copy(),
                "w2lp": w2lp, "w2r": w2r.astype(BF16),
                "b2e": b2e, "ones1": ones1, "iota": iota,
            }
        )

    res = run_bass_kernel_spmd(nc, in_maps, list(range(NCORES)), trace=trace)

    out = np.zeros((META, N, D), dtype=np.float32)
    slots_of = layout["slots_of"]
    for c in range(NCORES):
        oc = np.asarray(res.results[c]["out"])  # [META, 128, NSLOT*D]
        for g in range(META):
            # node (slot s, row p) = oc[g][p, s*D:(s+1)*D]
            per_node = oc[g].reshape(128, NSLOT, D).transpose(1, 0, 2)
            blocks = slots_of[g, c]
            rows = (blocks[:, None] * 128 + np.arange(128)[None, :]).reshape(-1)
            valid = rows < N
            out[g][rows[valid]] = per_node.reshape(NSH, D)[valid]

    # host fixup: zero-in-degree nodes (mean terms vanish; the kernel's
    # b2e fold subtracted colsum(W2l) unconditionally)
    for g in range(META):
        cnt = np.bincount(ei[g, 1], minlength=N)[:N]
        zn = np.nonzero(cnt == 0)[0]
        if len(zn):
            z = meta_x[g][zn] @ w1r[g] + b1[g]
            hz = np.where(z > 0, z, np.expm1(np.minimum(z, 0.0)))
            o = hz @ w2r[g] + b2[g]
            o = o - o.max(axis=1, keepdims=True)
            out[g][zn] = (o - np.log(np.exp(o).sum(axis=1, keepdims=True))
                          ).astype(np.float32)
    return out, res

